# revision 4
# baseline (speedup 1.0000x reference)
"""Trainium2 Bass kernel for nn_PosEmbeddingwithMask (gnn_message_passing).

reference semantics:
    pos_m          = where(mask, last_pred, pos)            # [N,3]
    extended_x     = x + MLP_p(pos_m)                       # [N,L]
    len[e]         = || pos_m[row[e]] - pos_m[col[e]] ||    # [E,1]
    extended_edge  = edge_attr + MLP_d(len)                 # [E,L]
with MLP(in) = relu(in @ W1 + b1) @ W2 + b2, L=128.

Strategy: 8-way data parallel (edges and nodes sharded across cores, pos/MLP
weights replicated).  Inside each core the edge path is chunked (2048 edges);
endpoint positions are gathered with indirect DMA from a masked-position DRAM
scratch, the 1->128->128 MLP runs on the PE (layer1 via masked K=32 weights so
every matmul streams from partition 0), and edge_attr is accumulated into the
result by the DMA engine itself (CCE add) so the big [E,128] tensor is only
ever streamed HBM->HBM.
"""

import math
from dataclasses import dataclass

import numpy as np

# ---------------------------------------------------------------- problem dims
N = 50000
E = 1600000
L = 128
NCORES = 8
P = 128
J = 16                  # edges per partition per chunk
CHUNK = P * J           # 2048 edges per chunk
G = J                   # groups of 128 edges per chunk
H = G // 2              # groups per psum half


def _ceil_to(a, b):
    return ((a + b - 1) // b) * b


@dataclass
class Cfg:
    npad: int           # padded full node count (multiple of 128)
    npc: int            # nodes per core (multiple of 128)
    epc: int            # edges per core (multiple of CHUNK)

    @property
    def n_chunks(self):
        return self.epc // CHUNK

    @property
    def n_node_tiles(self):
        return self.npc // P


def full_cfg() -> Cfg:
    npc = _ceil_to(math.ceil(N / NCORES), P)            # 6272
    epc = _ceil_to(math.ceil(E / NCORES), CHUNK)        # 200704
    npad = _ceil_to(N, P)                               # 50048 -> mult of 128
    # flat layout needs npad*3 divisible by 128 -> npad multiple of 128 ok
    return Cfg(npad=npad, npc=npc, epc=epc)


# ---------------------------------------------------------------- bass builder
def build(cfg: Cfg):
    import concourse.bacc as bacc
    import concourse.bass as bass
    import concourse.mybir as mybir
    import concourse.tile as tile
    from concourse.masks import make_identity

    dt = mybir.dt
    f32 = dt.float32
    Alu = mybir.AluOpType
    Act = mybir.ActivationFunctionType
    X = mybir.AxisListType.X

    npad, npc, epc = cfg.npad, cfg.npc, cfg.epc
    NCH = cfg.n_chunks
    NT = cfg.n_node_tiles
    A = npad // P           # flat cols per partition (per xyz component x3)

    nc = bacc.Bacc("TRN2", target_bir_lowering=False, debug=False)

    # ---------------- dram tensors (names are the in_map keys)
    pos_d = nc.dram_tensor("pos", [npad, 3], f32, kind="ExternalInput")
    lp_d = nc.dram_tensor("last_pred", [npad, 3], f32, kind="ExternalInput")
    mask_d = nc.dram_tensor("mask", [npad, 1], dt.uint8, kind="ExternalInput")
    posn_d = nc.dram_tensor("pos_n", [npc, 3], f32, kind="ExternalInput")
    lpn_d = nc.dram_tensor("lp_n", [npc, 3], f32, kind="ExternalInput")
    maskn_d = nc.dram_tensor("mask_n", [npc, 1], dt.uint8, kind="ExternalInput")
    x_d = nc.dram_tensor("x", [npc, L], f32, kind="ExternalInput")
    attr_d = nc.dram_tensor("edge_attr", [epc, L], f32, kind="ExternalInput")
    idx_d = nc.dram_tensor("idx", [NCH, P, 2 * J], dt.int32, kind="ExternalInput")
    wp1_d = nc.dram_tensor("Wp1", [3, L], f32, kind="ExternalInput")
    bp1_d = nc.dram_tensor("bp1", [1, L], f32, kind="ExternalInput")
    wp2_d = nc.dram_tensor("Wp2", [L, L], f32, kind="ExternalInput")
    bp2_d = nc.dram_tensor("bp2", [1, L], f32, kind="ExternalInput")
    wd1_d = nc.dram_tensor("Wd1", [1, L], f32, kind="ExternalInput")
    bd1_d = nc.dram_tensor("bd1", [1, L], f32, kind="ExternalInput")
    wd2_d = nc.dram_tensor("Wd2", [L, L], f32, kind="ExternalInput")
    bd2_d = nc.dram_tensor("bd2", [1, L], f32, kind="ExternalInput")
    extx_d = nc.dram_tensor("ext_x", [npc, L], f32, kind="ExternalOutput")
    exta_d = nc.dram_tensor("ext_attr", [epc, L], f32, kind="ExternalOutput")
    posm_d = nc.dram_tensor("pos_m", [npad, 3], f32, kind="Internal")

    with tile.TileContext(nc) as tc:
        with tc.tile_pool(name="const", bufs=1) as cp:
            # ---------------- constants / weights
            ident = cp.tile([P, P], f32, tag="ident")
            make_identity(nc, ident[:])

            wd1 = cp.tile([1, L], f32, tag="wd1")
            nc.sync.dma_start(wd1[:], wd1_d[:])
            bd1 = cp.tile([1, L], f32, tag="bd1")
            nc.sync.dma_start(bd1[:], bd1_d[:])
            wd2 = cp.tile([L, L], f32, tag="wd2")
            nc.sync.dma_start(wd2[:], wd2_d[:])
            wp1a = cp.tile([4, L], f32, tag="wp1a")
            nc.sync.dma_start(wp1a[0:3, :], wp1_d[:])
            nc.sync.dma_start(wp1a[3:4, :], bp1_d[:])
            wp2 = cp.tile([L, L], f32, tag="wp2")
            nc.sync.dma_start(wp2[:], wp2_d[:])

            # bd2 replicated [P, H*P]; bp2 replicated [P, P]
            bd2row = cp.tile([1, H * P], f32, tag="bd2row")
            for i in range(H):
                nc.sync.dma_start(bd2row[:, i * L:(i + 1) * L], bd2_d[:])
            bd2big = cp.tile([P, H * P], f32, tag="bd2big")
            nc.gpsimd.partition_broadcast(bd2big[:], bd2row[:])
            bp2row = cp.tile([1, L], f32, tag="bp2row")
            nc.sync.dma_start(bp2row[:], bp2_d[:])
            bp2rep = cp.tile([P, L], f32, tag="bp2rep")
            nc.gpsimd.partition_broadcast(bp2rep[:], bp2row[:])

            # masked layer-1 weights for the edge MLP: for group g the
            # contraction is over rhs_aug rows {g (len), J+g (ones)}.
            w1pads = []
            for g in range(G):
                t = cp.tile([2 * J, L], f32, tag=f"w1pad{g}")
                nc.vector.memset(t[:], 0.0)
                nc.sync.dma_start(t[g:g + 1, :], wd1_d[:])
                nc.sync.dma_start(t[J + g:J + g + 1, :], bd1_d[:])
                w1pads.append(t)

            # ---------------- phase 1: pos_m = pos + mask*(last_pred - pos)
            F3 = A * 3
            with tc.tile_pool(name="prep", bufs=1) as pp:
                posf = pp.tile([P, F3], f32, tag="posf")
                lpf = pp.tile([P, F3], f32, tag="lpf")
                msk8 = pp.tile([P, A], dt.uint8, tag="msk8")
                mskf = pp.tile([P, A], f32, tag="mskf")
                posflat = pos_d[:].rearrange("(p a) c -> p (a c)", p=P)
                nc.sync.dma_start(posf[:], posflat)
                nc.sync.dma_start(lpf[:], lp_d[:].rearrange("(p a) c -> p (a c)", p=P))
                nc.sync.dma_start(msk8[:], mask_d[:].rearrange("(p a) c -> p (a c)", p=P))
                nc.vector.tensor_copy(mskf[:], msk8[:])
                delta = pp.tile([P, F3], f32, tag="delta")
                nc.vector.tensor_tensor(delta[:], lpf[:], posf[:], op=Alu.subtract)
                dv = delta[:].rearrange("p (a c) -> p a c", c=3)
                for ci in range(3):
                    nc.vector.tensor_tensor(dv[:, :, ci], dv[:, :, ci], mskf[:], op=Alu.mult)
                nc.vector.tensor_tensor(delta[:], delta[:], posf[:], op=Alu.add)
                nc.sync.dma_start(posm_d[:].rearrange("(p a) c -> p (a c)", p=P), delta[:])

            # ---------------- phase 2: node path
            with tc.tile_pool(name="npsum", bufs=2, space="PSUM") as npp, \
                 tc.tile_pool(name="nsb", bufs=3) as nsp:
                for t in range(NT):
                    r0 = t * P
                    pos_t = nsp.tile([P, 3], f32, tag="npos")
                    nc.sync.dma_start(pos_t[:], posn_d[r0:r0 + P, :])
                    lp_t = nsp.tile([P, 3], f32, tag="nlp")
                    nc.sync.dma_start(lp_t[:], lpn_d[r0:r0 + P, :])
                    m8_t = nsp.tile([P, 1], dt.uint8, tag="nm8")
                    nc.sync.dma_start(m8_t[:], maskn_d[r0:r0 + P, :])
                    mf_t = nsp.tile([P, 1], f32, tag="nmf")
                    nc.vector.tensor_copy(mf_t[:], m8_t[:])
                    pos4 = nsp.tile([P, 4], f32, tag="npos4")
                    nc.gpsimd.memset(pos4[:], 1.0)
                    dl = nsp.tile([P, 3], f32, tag="ndl")
                    nc.vector.tensor_tensor(dl[:], lp_t[:], pos_t[:], op=Alu.subtract)
                    nc.vector.tensor_scalar_mul(dl[:], dl[:], mf_t[:])
                    nc.vector.tensor_tensor(pos4[:, 0:3], pos_t[:], dl[:], op=Alu.add)
                    # transpose [P,4] -> [4,P]
                    ps_t4 = npp.tile([4, P], f32, tag="npt")
                    nc.tensor.transpose(ps_t4[:], pos4[:], ident[:])
                    rhs4 = nsp.tile([4, P], f32, tag="nrhs")
                    nc.scalar.copy(rhs4[:], ps_t4[:])
                    pre1n = npp.tile([P, P], f32, tag="npre")
                    nc.tensor.matmul(pre1n[:], wp1a[:], rhs4[:], start=True, stop=True)
                    h1n = nsp.tile([P, P], f32, tag="nh1")
                    nc.scalar.activation(h1n[:], pre1n[:], Act.Relu)
                    x_t = nsp.tile([P, L], f32, tag="nx")
                    nc.sync.dma_start(x_t[:], x_d[r0:r0 + P, :])
                    yn = npp.tile([P, P], f32, tag="nyn")
                    nc.tensor.matmul(yn[:], h1n[:], wp2[:], start=True, stop=False)
                    nc.tensor.matmul(yn[:], ident[:], x_t[:], start=False, stop=True)
                    outx = nsp.tile([P, L], f32, tag="noutx")
                    nc.vector.tensor_tensor(outx[:], yn[:], bp2rep[:], op=Alu.add)
                    nc.sync.dma_start(extx_d[r0:r0 + P, :], outx[:])

            # ---------------- phase 3: edge path
            attr_v = attr_d[:].rearrange("(c p j) l -> c p (j l)", p=P, j=J)
            exta_v = exta_d[:].rearrange("(c p j) l -> c p (j l)", p=P, j=J)
            idx_v = idx_d[:]
            with tc.tile_pool(name="pre1", bufs=1, space="PSUM") as pre1p, \
                 tc.tile_pool(name="ypsum", bufs=1, space="PSUM") as ypp, \
                 tc.tile_pool(name="esb", bufs=3) as ep, \
                 tc.tile_pool(name="ebig", bufs=2) as bp:
                for c in range(NCH):
                    idx_t = ep.tile([P, 2 * J], dt.int32, tag="idx")
                    nc.sync.dma_start(idx_t[:], idx_v[c])
                    # one [P,1]-offset indirect DMA per endpoint column (the
                    # only offset form the HW DGE honours): cols 0:J are row
                    # endpoints, cols J:2J are col endpoints
                    gr = ep.tile([P, J * 3], f32, tag="gr")
                    gc = ep.tile([P, J * 3], f32, tag="gc")
                    for k in range(J):
                        nc.gpsimd.indirect_dma_start(
                            out=gr[:, 3 * k:3 * k + 3],
                            out_offset=None,
                            in_=posm_d[:],
                            in_offset=bass.IndirectOffsetOnAxis(ap=idx_t[:, k:k + 1], axis=0),
                        )
                        nc.gpsimd.indirect_dma_start(
                            out=gc[:, 3 * k:3 * k + 3],
                            out_offset=None,
                            in_=posm_d[:],
                            in_offset=bass.IndirectOffsetOnAxis(ap=idx_t[:, J + k:J + k + 1], axis=0),
                        )
                    d_t = ep.tile([P, J * 3], f32, tag="d")
                    nc.vector.tensor_tensor(d_t[:], gr[:], gc[:], op=Alu.subtract)
                    dd_t = ep.tile([P, J * 3], f32, tag="dd")
                    nc.vector.tensor_tensor(dd_t[:], d_t[:], d_t[:], op=Alu.mult)
                    lsq = ep.tile([P, J], f32, tag="lsq")
                    nc.vector.tensor_reduce(
                        lsq[:], dd_t[:].rearrange("p (j c) -> p j c", c=3), X, Alu.add)
                    rhs_pre = ep.tile([P, 2 * J], f32, tag="rhspre")
                    nc.gpsimd.memset(rhs_pre[:], 1.0)
                    nc.scalar.sqrt(rhs_pre[:, 0:J], lsq[:])

                    preA = pre1p.tile([P, H * P], f32, tag="preA")
                    preB = pre1p.tile([P, H * P], f32, tag="preB")
                    # transpose lengths+ones into k-major, staged via preA's bank
                    nc.tensor.transpose(preA[0:2 * J, 0:P], rhs_pre[:], ident[:])
                    rhs_aug = ep.tile([2 * J, P], f32, tag="rhsaug")
                    nc.scalar.copy(rhs_aug[:], preA[0:2 * J, 0:P])

                    # layer 1: pre[h, e] = Wd1[h]*len[e] + bd1[h]
                    for g in range(G):
                        pre = preA if g < H else preB
                        col = (g % H) * P
                        nc.tensor.matmul(pre[:, col:col + P], w1pads[g][:], rhs_aug[:],
                                         start=True, stop=True)
                    h1 = bp.tile([P, G * P], f32, tag="h1")
                    nc.scalar.activation(h1[:, 0:H * P], preA[:], Act.Relu)
                    nc.scalar.activation(h1[:, H * P:], preB[:], Act.Relu)

                    # layer 2: y[e, o] = sum_h h1[h, e] * Wd2[h, o]
                    yA = ypp.tile([P, H * P], f32, tag="yA")
                    yB = ypp.tile([P, H * P], f32, tag="yB")
                    for g in range(G):
                        y = yA if g < H else yB
                        col = (g % H) * P
                        nc.tensor.matmul(y[:, col:col + P], h1[:, g * P:(g + 1) * P],
                                         wd2[:], start=True, stop=True)
                    out_sb = bp.tile([P, G * P], f32, tag="osb")
                    nc.vector.tensor_tensor(out_sb[:, 0:H * P], yA[:], bd2big[:], op=Alu.add)
                    nc.vector.tensor_tensor(out_sb[:, H * P:], yB[:], bd2big[:], op=Alu.add)
                    # edge_attr accumulated by the DMA engine (CCE add)
                    nc.gpsimd.dma_start(out_sb[:], attr_v[c], accum_op=Alu.add)
                    nc.sync.dma_start(exta_v[c], out_sb[:])

    nc.compile()
    return nc


# ---------------------------------------------------------------- host wrapper
def prep_inputs(cfg: Cfg, pos, x, edge_attr, edge_index, last_pred, mask_idx,
                Wp1, bp1, Wp2, bp2, Wd1, bd1, Wd2, bd2):
    """Pad + shard the full problem into per-core in_maps."""
    f32 = np.float32
    npad, npc, epc = cfg.npad, cfg.npc, cfg.epc
    n = pos.shape[0]
    e = edge_attr.shape[0]

    pos_p = np.zeros((npad, 3), f32)
    pos_p[:n] = np.asarray(pos, f32)
    lp_p = np.zeros((npad, 3), f32)
    lp_p[:n] = np.asarray(last_pred, f32)
    mask_p = np.zeros((npad, 1), np.uint8)
    mask_p[:n] = np.asarray(mask_idx).astype(np.uint8).reshape(n, 1)

    ntot = npc * NCORES
    posn_p = np.zeros((ntot, 3), f32)
    posn_p[:n] = np.asarray(pos, f32)
    lpn_p = np.zeros((ntot, 3), f32)
    lpn_p[:n] = np.asarray(last_pred, f32)
    maskn_p = np.zeros((ntot, 1), np.uint8)
    maskn_p[:n] = np.asarray(mask_idx).astype(np.uint8).reshape(n, 1)
    x_p = np.zeros((ntot, L), f32)
    x_p[:n] = np.asarray(x, f32)

    etot = epc * NCORES
    ea_p = np.zeros((etot, L), f32)
    ea_p[:e] = np.asarray(edge_attr, f32)
    row_p = np.zeros(etot, np.int32)
    col_p = np.zeros(etot, np.int32)
    row_p[:e] = np.asarray(edge_index[0], np.int64).astype(np.int32)
    col_p[:e] = np.asarray(edge_index[1], np.int64).astype(np.int32)
    nch = cfg.n_chunks
    r4 = row_p.reshape(NCORES, nch, P, J)
    c4 = col_p.reshape(NCORES, nch, P, J)
    idx_il = np.concatenate([r4, c4], axis=-1)   # cols 0:J rows, J:2J cols
    idx_il = np.ascontiguousarray(idx_il, np.int32)

    w = dict(
        Wp1=np.asarray(Wp1, f32).reshape(3, L),
        bp1=np.asarray(bp1, f32).reshape(1, L),
        Wp2=np.asarray(Wp2, f32).reshape(L, L),
        bp2=np.asarray(bp2, f32).reshape(1, L),
        Wd1=np.asarray(Wd1, f32).reshape(1, L),
        bd1=np.asarray(bd1, f32).reshape(1, L),
        Wd2=np.asarray(Wd2, f32).reshape(L, L),
        bd2=np.asarray(bd2, f32).reshape(1, L),
    )

    in_maps = []
    for cidx in range(NCORES):
        in_maps.append(dict(
            pos=pos_p,
            last_pred=lp_p,
            mask=mask_p,
            pos_n=np.ascontiguousarray(posn_p[cidx * npc:(cidx + 1) * npc]),
            lp_n=np.ascontiguousarray(lpn_p[cidx * npc:(cidx + 1) * npc]),
            mask_n=np.ascontiguousarray(maskn_p[cidx * npc:(cidx + 1) * npc]),
            x=np.ascontiguousarray(x_p[cidx * npc:(cidx + 1) * npc]),
            edge_attr=np.ascontiguousarray(ea_p[cidx * epc:(cidx + 1) * epc]),
            idx=np.ascontiguousarray(idx_il[cidx]),
            **w,
        ))
    return in_maps


_BUILT = {}


def get_built(cfg: Cfg):
    key = (cfg.npad, cfg.npc, cfg.epc)
    if key not in _BUILT:
        _BUILT[key] = build(cfg)
    return _BUILT[key]


def run(cfg: Cfg, inputs, trace=False):
    from concourse import bass_utils
    nc = get_built(cfg)
    in_maps = prep_inputs(cfg, **inputs)
    res = bass_utils.run_bass_kernel_spmd(
        nc, in_maps, core_ids=list(range(NCORES)), trace=trace)
    n = inputs["pos"].shape[0]
    e = inputs["edge_attr"].shape[0]
    ext_x = np.concatenate([res.results[i]["ext_x"] for i in range(NCORES)])[:n]
    ext_attr = np.concatenate([res.results[i]["ext_attr"] for i in range(NCORES)])[:e]
    return (ext_x, ext_attr), res


def kernel(**inputs):
    inputs = {k: np.asarray(v) for k, v in inputs.items()}
    (ext_x, ext_attr), _ = run(full_cfg(), inputs)
    return ext_x, ext_attr
